# revision 9
# baseline (speedup 1.0000x reference)
"""Trainium2 Bass kernel for nn_CellDecoder (span-pool + ffnn + biaffine pairs).

Strategy: head_idx/tail_idx only reference E=256 entities, so instead of
computing the biaffine per pair (P=65536), the cores build the full E x E
biaffine logit table (small matmuls). The per-pair work is a pure table
lookup with host-known indices, done during the host-side unshard/assembly
step, so the device kernel ships the dense table.

Sharding: 8 cores = batch (2) x e1-half (2) x output-logit o (2). The
tail ffnn chain (table columns) is replicated; the head chain + biaffine
run only on each core's 128 table rows and its o. Per-core "which rows"
is steered purely through the inputs: each core receives its batch's
mask/embedding columns rotated so its 128 head entities land in columns
0:128 (the SPMD program is identical on all cores; the host un-rotates
column indices during assembly).

Perf notes:
- Everything is bf16; rel err ~5e-3, well under the 2e-2 gate.
- DMA instruction issue costs ~600ns each on a ring, so bulk tensors go
  out as few large transfers on the sync ring in exact consumption order;
  first-needed operands (mask, label-emb rows) ride the scalar ring.
- The label-embedding half of ent_repr is a tiny host-side lookup table
  -> shipped as data straight into the entT tile instead of matmuls.
- Matmul loops are kt-outer over chunk boundaries; psum->sbuf copies
  alternate vector/scalar engines.
"""

import os

os.environ.setdefault("JAX_PLATFORMS", "axon,cpu")

import numpy as np
import ml_dtypes

import concourse.bass as bass
import concourse.tile as tile
from concourse import bacc, mybir
from concourse.bass_utils import run_bass_kernel_spmd

dt = mybir.dt

B, T, D, E, P = 2, 512, 768, 256, 65536
MLP = 2 * D  # 1536
H1, H2 = MLP // 2, MLP // 4  # 768, 384
NL = 5
OUT = 2
N_CORES = 8
EH = 128  # head rows per core

KT_MLP = MLP // 128  # 12
KT_H1 = H1 // 128  # 6
KT_H2 = H2 // 128  # 3
KT_T = T // 128  # 4
MT_D = D // 128  # 6
MT_H1 = H1 // 128  # 6
MT_H2 = H2 // 128  # 3

SF_COLS = 2 * MT_H1 + 2 * MT_H2 + 1  # b1h, b1t, b2h, b2t, blin_o = 19

_cache: dict = {}


def _build(ni: int = 0):
    """Build + compile the SPMD program (ni unused, kept for test.py interface)."""
    if 0 in _cache:
        return _cache[0]

    nc = bacc.Bacc("TRN2", target_bir_lowering=False, debug=False, num_devices=N_CORES)

    f32 = dt.float32
    bf16 = dt.bfloat16

    # [128, cols] host-packed operand tensors
    d_hs = nc.dram_tensor("hs", [128, KT_T * D], bf16, kind="ExternalInput")
    d_maskn = nc.dram_tensor("masknT", [128, KT_T * E], bf16, kind="ExternalInput")
    # label-emb rows of entT (host-computed lookup) | ones row
    d_emb = nc.dram_tensor("embT", [128, MT_D * E + E], bf16, kind="ExternalInput")
    d_wh1 = nc.dram_tensor("Wh1", [128, KT_MLP * H1], bf16, kind="ExternalInput")
    d_wt1 = nc.dram_tensor("Wt1", [128, KT_MLP * H1], bf16, kind="ExternalInput")
    d_wh2 = nc.dram_tensor("Wh2", [128, KT_H1 * H2], bf16, kind="ExternalInput")
    d_wt2 = nc.dram_tensor("Wt2", [128, KT_H1 * H2], bf16, kind="ExternalInput")
    # this core's Wbil_o | wlin col o
    d_wtl = nc.dram_tensor(
        "Wtl", [128, KT_H2 * H2 + 2 * KT_H2], bf16, kind="ExternalInput"
    )
    d_smf = nc.dram_tensor("smf", [128, SF_COLS], f32, kind="ExternalInput")
    # output: this core's 128 table rows, columns in rotated entity order
    d_slab = nc.dram_tensor("slab", [128, E], bf16, kind="ExternalOutput")

    with tile.TileContext(nc) as tc:
        with (
            tc.tile_pool(name="wbig", bufs=1) as wbig,
            tc.tile_pool(name="wsml", bufs=1) as wsml,
            tc.tile_pool(name="act", bufs=1) as act,
            tc.tile_pool(name="ps", bufs=6, space="PSUM") as ps,
            tc.tile_pool(name="ps1", bufs=2, space="PSUM") as ps1,
        ):
            # ---- first-needed operands on the scalar ring, chunked per kt
            #      so pooling kt=0 starts as soon as the first slices land ----
            maskn = wsml.tile([128, KT_T, E], bf16, tag="maskn", name="maskn")
            maskn_src = d_maskn.ap().rearrange("p (kt n) -> p kt n", kt=KT_T)
            for kt in range(KT_T):
                nc.scalar.dma_start(
                    maskn[:, kt : kt + 1, :], maskn_src[:, kt : kt + 1, :]
                )
            # entT rows 0:6 filled by pooling below; rows 6:12 + ones by DMA
            entT = act.tile([128, KT_MLP, E], bf16, tag="entT")
            emb_src = d_emb.ap().rearrange("p (kt n) -> p kt n", kt=MT_D + 1)
            nc.scalar.dma_start(entT[:, MT_D:KT_MLP, :], emb_src[:, 0:MT_D, :])
            ones_t = wsml.tile([1, E], bf16, tag="ones", name="ones_t")
            nc.scalar.dma_start(ones_t[:], emb_src[0:1, MT_D, :])
            smf = wsml.tile([128, SF_COLS], f32, tag="smf", name="smf")
            nc.scalar.dma_start(smf[:], d_smf.ap())

            b1 = {"h": smf[:, 0:MT_H1], "t": smf[:, MT_H1 : 2 * MT_H1]}
            b2 = {
                "h": smf[:, 2 * MT_H1 : 2 * MT_H1 + MT_H2],
                "t": smf[:, 2 * MT_H1 + MT_H2 : 2 * MT_H1 + 2 * MT_H2],
            }
            blin = smf[0:1, SF_COLS - 1 : SF_COLS]

            # ---- bulk on the sync ring in consumption order ----
            def loadb(pool, name, dram, kt, n, nchunks=1):
                t = pool.tile([128, kt, n], bf16, tag=name, name=name)
                src = dram.ap().rearrange("p (kt n) -> p kt n", kt=kt)
                step = kt // nchunks
                for k0 in range(0, kt, step):
                    nc.sync.dma_start(t[:, k0 : k0 + step, :], src[:, k0 : k0 + step, :])
                return t

            hs = loadb(wbig, "hs", d_hs, KT_T, D, nchunks=4)
            w1 = {"h": loadb(wbig, "w1h", d_wh1, KT_MLP, H1, nchunks=2)}
            w2 = {"h": loadb(wbig, "w2h", d_wh2, KT_H1, H2)}
            wtl = wsml.tile([128, KT_H2, H2 + 2], bf16, tag="wtl", name="wtl")
            nc.sync.dma_start(
                wtl[:, :, 0:H2],
                d_wtl.ap()[:, 0 : KT_H2 * H2].rearrange("p (kt n) -> p kt n", kt=KT_H2),
            )
            nc.sync.dma_start(
                wtl[:, :, H2 : H2 + 2],
                d_wtl.ap()[:, KT_H2 * H2 :].rearrange("p (kt n) -> p kt n", kt=KT_H2),
            )
            wb_o = wtl[:, :, 0:H2]
            wlin = wtl[:, :, H2 : H2 + 2]  # [:, kt, 0]=head col o, [:, kt, 1]=tail
            w1["t"] = loadb(wbig, "w1t", d_wt1, KT_MLP, H1, nchunks=2)
            w2["t"] = loadb(wbig, "w2t", d_wt2, KT_H1, H2)

            # copy engines alternate to halve serial copy chains
            def copy(i, dst, src):
                if i % 2:
                    nc.scalar.activation(
                        dst, src, mybir.ActivationFunctionType.Identity
                    )
                else:
                    nc.vector.tensor_copy(dst, src)

            # ---- pooled^T -> entT rows 0:6  (kt-outer over hs chunks) ----
            pool_ps = [
                ps.tile([128, E], f32, tag="mm", name=f"pp{m}") for m in range(MT_D)
            ]
            for kt in range(KT_T):
                for mt in range(MT_D):
                    nc.tensor.matmul(
                        pool_ps[mt][:],
                        hs[:, kt, mt * 128 : (mt + 1) * 128],
                        maskn[:, kt, :],
                        start=(kt == 0),
                        stop=(kt == KT_T - 1),
                    )
            for mt in range(MT_D):
                copy(mt, entT[:, mt, :], pool_ps[mt][:])

            # ---- ffnn chains; head (cols 0:EH) first, tail full E ----
            h2T = {}

            def ffnn(side):
                n = EH if side == "h" else E
                h1T = act.tile(
                    [128, KT_H1, n], bf16, tag=f"h1T{side}", name=f"h1T{side}"
                )
                accs = [
                    ps.tile([128, n], f32, tag="mm", name=f"l1{side}{m}")
                    for m in range(MT_H1)
                ]
                for kt in range(KT_MLP):
                    for mt in range(MT_H1):
                        nc.tensor.matmul(
                            accs[mt][:],
                            w1[side][:, kt, mt * 128 : (mt + 1) * 128],
                            entT[:, kt, 0:n],
                            start=(kt == 0),
                            stop=(kt == KT_MLP - 1),
                        )
                for mt in range(MT_H1):
                    nc.scalar.activation(
                        h1T[:, mt, :],
                        accs[mt][:],
                        mybir.ActivationFunctionType.Relu,
                        bias=b1[side][:, mt : mt + 1],
                    )
                h2T[side] = act.tile(
                    [128, KT_H2, n], bf16, tag=f"h2T{side}", name=f"h2T{side}"
                )
                accs2 = [
                    ps.tile([128, n], f32, tag="mm", name=f"l2{side}{m}")
                    for m in range(MT_H2)
                ]
                for kt in range(KT_H1):
                    for mt in range(MT_H2):
                        nc.tensor.matmul(
                            accs2[mt][:],
                            w2[side][:, kt, mt * 128 : (mt + 1) * 128],
                            h1T[:, kt, :],
                            start=(kt == 0),
                            stop=(kt == KT_H1 - 1),
                        )
                for mt in range(MT_H2):
                    nc.scalar.activation(
                        h2T[side][:, mt, :],
                        accs2[mt][:],
                        mybir.ActivationFunctionType.Relu,
                        bias=b2[side][:, mt : mt + 1],
                    )

            ffnn("h")

            # ---- N_o^T [H2, EH] and linh [1, EH] for this core's o ----
            nTo = act.tile([128, KT_H2, EH], bf16, tag="nTo", name="nTo")
            accs = [
                ps.tile([128, EH], f32, tag="mm", name=f"nt{m}") for m in range(MT_H2)
            ]
            for kt in range(KT_H2):
                for mt in range(MT_H2):
                    nc.tensor.matmul(
                        accs[mt][:],
                        wb_o[:, kt, mt * 128 : (mt + 1) * 128],
                        h2T["h"][:, kt, :],
                        start=(kt == 0),
                        stop=(kt == KT_H2 - 1),
                    )
            for mt in range(MT_H2):
                copy(mt, nTo[:, mt, :], accs[mt][:])

            linh = act.tile([1, EH], bf16, tag="linh", name="linh")
            p = ps1.tile([1, EH], f32, tag="lin")
            for kt in range(KT_H2):
                nc.tensor.matmul(
                    p[:],
                    wlin[:, kt, 0:1],
                    h2T["h"][:, kt, :],
                    start=(kt == 0),
                    stop=(kt == KT_H2 - 1),
                )
            nc.vector.tensor_copy(linh[:], p[:])

            ffnn("t")

            lint = act.tile([1, E], bf16, tag="lint", name="lint")
            p = ps1.tile([1, E], f32, tag="lin")
            for kt in range(KT_H2):
                nc.tensor.matmul(
                    p[:],
                    wlin[:, kt, 1:2],
                    h2T["t"][:, kt, :],
                    start=(kt == 0),
                    stop=(kt == KT_H2 - 1),
                )
            # + b_lin[o] folded in via bias
            nc.scalar.activation(
                lint[:],
                p[:],
                mybir.ActivationFunctionType.Identity,
                bias=blin,
            )

            # ---- table rows for this core: [128, E] ----
            slab = act.tile([128, E], bf16, tag="slab")
            p = ps.tile([128, E], f32, tag="mm")
            for kt in range(KT_H2):
                nc.tensor.matmul(
                    p[:],
                    nTo[:, kt, :],
                    h2T["t"][:, kt, :],
                    start=(kt == 0),
                    stop=False,
                )
            nc.tensor.matmul(p[:], linh[:], ones_t[:], start=False, stop=False)
            nc.tensor.matmul(
                p[:], ones_t[:, 0:128], lint[:], start=False, stop=True
            )
            nc.vector.tensor_copy(slab[:], p[:])
            nc.sync.dma_start(d_slab.ap(), slab[:])

    nc.compile()
    _cache[0] = nc
    return nc


def _pack(w, kt, dtype=ml_dtypes.bfloat16):
    """[kt*128, n] row-major -> [128, kt*n] partition-packed."""
    n = w.shape[1]
    return np.ascontiguousarray(
        w.reshape(kt, 128, n).transpose(1, 0, 2).reshape(128, kt * n).astype(dtype)
    )


def _prep_host(inputs):
    """Host-side input packing -> per-core in_maps + assembly info."""
    hs = np.asarray(inputs["hidden_states"], dtype=np.float32)
    start = np.asarray(inputs["entity_start"]).astype(np.int64)
    end = np.asarray(inputs["entity_end"]).astype(np.int64)
    label = np.asarray(inputs["entity_label"]).astype(np.int64)

    t = np.arange(T)
    mask = (
        (t[None, None, :] >= start[:, :, None]) & (t[None, None, :] < end[:, :, None])
    ).astype(np.float32)  # [B,E,T]
    counts = np.maximum(mask.sum(-1, keepdims=True), 1.0)
    masknT = (mask / counts).transpose(0, 2, 1)  # [B,T,E]

    def f32(x):
        return np.ascontiguousarray(np.asarray(x, dtype=np.float32))

    bf = ml_dtypes.bfloat16
    w_bil = f32(inputs["W_bil"])
    w_lin = f32(inputs["W_lin"])
    b_lin = f32(inputs["b_lin"])
    emb_all = f32(inputs["entity_emb_w"])

    shared = {
        "Wh1": _pack(f32(inputs["Wh1"]), KT_MLP),
        "Wt1": _pack(f32(inputs["Wt1"]), KT_MLP),
        "Wh2": _pack(f32(inputs["Wh2"]), KT_H1),
        "Wt2": _pack(f32(inputs["Wt2"]), KT_H1),
    }

    # per-o: Wbil_o | [wlin head col o, wlin tail col o]
    wtl_o = []
    for o in range(OUT):
        wb = _pack(w_bil[o], KT_H2, np.float32)  # [128, 3*384]
        wl = np.stack(
            [
                w_lin[:H2, o].reshape(KT_H2, 128).T,  # [128, 3] head col
                w_lin[H2:, o].reshape(KT_H2, 128).T,  # [128, 3] tail col
            ],
            axis=2,
        ).reshape(128, 2 * KT_H2)
        wtl_o.append(
            np.ascontiguousarray(
                np.concatenate([wb, wl], axis=1).astype(bf)
            )
        )

    smf_o = []
    for o in range(OUT):
        smf = np.zeros((128, SF_COLS), np.float32)
        smf[:, 0:MT_H1] = f32(inputs["bh1"]).reshape(MT_H1, 128).T
        smf[:, MT_H1 : 2 * MT_H1] = f32(inputs["bt1"]).reshape(MT_H1, 128).T
        smf[:, 2 * MT_H1 : 2 * MT_H1 + MT_H2] = (
            f32(inputs["bh2"]).reshape(MT_H2, 128).T
        )
        smf[:, 2 * MT_H1 + MT_H2 : 2 * MT_H1 + 2 * MT_H2] = (
            f32(inputs["bt2"]).reshape(MT_H2, 128).T
        )
        smf[0, SF_COLS - 1] = b_lin[o]
        smf_o.append(smf)

    in_maps = []
    for i in range(N_CORES):
        b, q = divmod(i, 4)
        m, o = divmod(q, 2)
        rot = (np.arange(E) + EH * m) % E  # rotated entity order
        mrot = np.ascontiguousarray(masknT[b][:, rot])
        embT = emb_all[label[b][rot]].T  # [D, E] in rotated order
        embp = np.zeros((128, (MT_D + 1) * E), np.float32)
        embp[:, 0 : MT_D * E] = _pack(embT, MT_D, np.float32)
        embp[0, MT_D * E :] = 1.0  # ones row
        mm = dict(shared)
        mm["hs"] = _pack(hs[b], KT_T)
        mm["masknT"] = _pack(mrot, KT_T)
        mm["embT"] = embp.astype(bf)
        mm["Wtl"] = wtl_o[o]
        mm["smf"] = smf_o[o]
        in_maps.append(mm)

    head_idx = np.asarray(inputs["head_idx"]).astype(np.int64)
    tail_idx = np.asarray(inputs["tail_idx"]).astype(np.int64)
    return in_maps, (head_idx, tail_idx), 0


def kernel(**inputs) -> np.ndarray:
    in_maps, (head_idx, tail_idx), ni = _prep_host(inputs)
    nc = _build(ni)
    res = run_bass_kernel_spmd(nc, in_maps, list(range(N_CORES)))
    out = np.zeros((B, P, OUT), np.float32)
    for b in range(B):
        slabs = np.stack(
            [res.results[4 * b + q]["slab"].astype(np.float32) for q in range(4)]
        )  # [q, 128, E]; q = 2*m + o
        e1, e2 = head_idx[b], tail_idx[b]
        m = e1 // EH
        p_ = e1 % EH
        col = (e2 - EH * m) % E
        for o in range(OUT):
            out[b, :, o] = slabs[2 * m + o, p_, col]
    return out


# revision 11
# speedup vs baseline: 1.0816x; 1.0816x over previous
"""Trainium2 Bass kernel for nn_CellDecoder (span-pool + ffnn + biaffine pairs).

Strategy: head_idx/tail_idx only reference E=256 entities, so instead of
computing the biaffine per pair (P=65536), the cores build the full E x E
biaffine logit table (small matmuls). The per-pair work is a pure table
lookup with host-known indices, done during the host-side unshard/assembly
step, so the device kernel ships the dense table.

Sharding: 8 cores = batch (2) x e1-half (2) x output-logit o (2). The
tail ffnn chain (table columns) is replicated; the head chain + biaffine
run only on each core's 128 table rows and its o. Per-core "which rows"
is steered purely through the inputs: each core receives its batch's
mask/embedding columns rotated so its 128 head entities land in columns
0:128 (the SPMD program is identical on all cores; the host un-rotates
column indices during assembly).

Perf notes:
- Everything is bf16; rel err ~5e-3, well under the 2e-2 gate.
- DMA instruction issue costs ~600ns each on a ring, so bulk tensors go
  out as few large transfers on the sync ring in exact consumption order;
  first-needed operands (mask, label-emb rows) ride the scalar ring.
- The label-embedding half of ent_repr is a tiny host-side lookup table
  -> shipped as data straight into the entT tile instead of matmuls.
- Matmul loops are kt-outer over chunk boundaries; psum->sbuf copies
  alternate vector/scalar engines.
"""

import os

os.environ.setdefault("JAX_PLATFORMS", "axon,cpu")

import numpy as np
import ml_dtypes

import concourse.bass as bass
import concourse.tile as tile
from concourse import bacc, mybir
from concourse.bass_utils import run_bass_kernel_spmd

dt = mybir.dt

B, T, D, E, P = 2, 512, 768, 256, 65536
MLP = 2 * D  # 1536
H1, H2 = MLP // 2, MLP // 4  # 768, 384
NL = 5
OUT = 2
N_CORES = 8
EH = 128  # head rows per core

KT_MLP = MLP // 128  # 12
KT_H1 = H1 // 128  # 6
KT_H2 = H2 // 128  # 3
KT_T = T // 128  # 4
MT_D = D // 128  # 6
MT_H1 = H1 // 128  # 6
MT_H2 = H2 // 128  # 3

SF_COLS = 2 * MT_H1 + 2 * MT_H2 + 1  # b1h, b1t, b2h, b2t, blin_o = 19

_cache: dict = {}


def _build(ni: int = 0):
    """Build + compile the SPMD program (ni unused, kept for test.py interface)."""
    if 0 in _cache:
        return _cache[0]

    nc = bacc.Bacc("TRN2", target_bir_lowering=False, debug=False, num_devices=N_CORES)

    f32 = dt.float32
    bf16 = dt.bfloat16

    # [128, cols] host-packed operand tensors
    d_hs = nc.dram_tensor("hs", [128, KT_T * D], bf16, kind="ExternalInput")
    d_maskn = nc.dram_tensor("masknT", [128, KT_T * E], bf16, kind="ExternalInput")
    # label-emb rows of entT (host-computed lookup) | ones row
    d_emb = nc.dram_tensor("embT", [128, MT_D * E + E], bf16, kind="ExternalInput")
    d_wh1 = nc.dram_tensor("Wh1", [128, KT_MLP * H1], bf16, kind="ExternalInput")
    d_wt1 = nc.dram_tensor("Wt1", [128, KT_MLP * H1], bf16, kind="ExternalInput")
    d_wh2 = nc.dram_tensor("Wh2", [128, KT_H1 * H2], bf16, kind="ExternalInput")
    d_wt2 = nc.dram_tensor("Wt2", [128, KT_H1 * H2], bf16, kind="ExternalInput")
    # this core's Wbil_o | wlin col o
    d_wtl = nc.dram_tensor(
        "Wtl", [128, KT_H2 * H2 + 2 * KT_H2], bf16, kind="ExternalInput"
    )
    d_smf = nc.dram_tensor("smf", [128, SF_COLS], f32, kind="ExternalInput")
    # output: this core's 128 table rows, columns in rotated entity order
    d_slab = nc.dram_tensor("slab", [128, E], bf16, kind="ExternalOutput")

    with tile.TileContext(nc) as tc:
        with (
            tc.tile_pool(name="wbig", bufs=1) as wbig,
            tc.tile_pool(name="wsml", bufs=1) as wsml,
            tc.tile_pool(name="act", bufs=1) as act,
            tc.tile_pool(name="ps", bufs=6, space="PSUM") as ps,
            tc.tile_pool(name="ps1", bufs=2, space="PSUM") as ps1,
        ):
            # ---- first-needed operands on the scalar ring ----
            maskn = wsml.tile([128, KT_T, E], bf16, tag="maskn", name="maskn")
            nc.scalar.dma_start(
                maskn[:], d_maskn.ap().rearrange("p (kt n) -> p kt n", kt=KT_T)
            )
            # entT rows 0:6 filled by pooling below; rows 6:12 + ones by DMA
            entT = act.tile([128, KT_MLP, E], bf16, tag="entT")
            emb_src = d_emb.ap().rearrange("p (kt n) -> p kt n", kt=MT_D + 1)
            nc.scalar.dma_start(entT[:, MT_D:KT_MLP, :], emb_src[:, 0:MT_D, :])
            ones_t = wsml.tile([1, E], bf16, tag="ones", name="ones_t")
            nc.scalar.dma_start(ones_t[:], emb_src[0:1, MT_D, :])
            smf = wsml.tile([128, SF_COLS], f32, tag="smf", name="smf")
            nc.scalar.dma_start(smf[:], d_smf.ap())

            b1 = {"h": smf[:, 0:MT_H1], "t": smf[:, MT_H1 : 2 * MT_H1]}
            b2 = {
                "h": smf[:, 2 * MT_H1 : 2 * MT_H1 + MT_H2],
                "t": smf[:, 2 * MT_H1 + MT_H2 : 2 * MT_H1 + 2 * MT_H2],
            }
            blin = smf[0:1, SF_COLS - 1 : SF_COLS]

            # ---- bulk on the sync ring in consumption order ----
            def loadb(pool, name, dram, kt, n, nchunks=1):
                t = pool.tile([128, kt, n], bf16, tag=name, name=name)
                src = dram.ap().rearrange("p (kt n) -> p kt n", kt=kt)
                step = kt // nchunks
                for k0 in range(0, kt, step):
                    nc.sync.dma_start(t[:, k0 : k0 + step, :], src[:, k0 : k0 + step, :])
                return t

            hs = loadb(wbig, "hs", d_hs, KT_T, D, nchunks=2)
            w1 = {"h": loadb(wbig, "w1h", d_wh1, KT_MLP, H1, nchunks=2)}
            w2 = {"h": loadb(wbig, "w2h", d_wh2, KT_H1, H2)}
            wtl = wsml.tile([128, KT_H2, H2 + 2], bf16, tag="wtl", name="wtl")
            nc.sync.dma_start(
                wtl[:, :, 0:H2],
                d_wtl.ap()[:, 0 : KT_H2 * H2].rearrange("p (kt n) -> p kt n", kt=KT_H2),
            )
            nc.sync.dma_start(
                wtl[:, :, H2 : H2 + 2],
                d_wtl.ap()[:, KT_H2 * H2 :].rearrange("p (kt n) -> p kt n", kt=KT_H2),
            )
            wb_o = wtl[:, :, 0:H2]
            wlin = wtl[:, :, H2 : H2 + 2]  # [:, kt, 0]=head col o, [:, kt, 1]=tail
            w1["t"] = loadb(wbig, "w1t", d_wt1, KT_MLP, H1, nchunks=2)
            w2["t"] = loadb(wbig, "w2t", d_wt2, KT_H1, H2)

            # copy engines alternate to halve serial copy chains
            def copy(i, dst, src):
                if i % 2:
                    nc.scalar.activation(
                        dst, src, mybir.ActivationFunctionType.Identity
                    )
                else:
                    nc.vector.tensor_copy(dst, src)

            # ---- pooled^T -> entT rows 0:6  (kt-outer over hs chunks) ----
            pool_ps = [
                ps.tile([128, E], f32, tag="mm", name=f"pp{m}") for m in range(MT_D)
            ]
            for kt in range(KT_T):
                for mt in range(MT_D):
                    nc.tensor.matmul(
                        pool_ps[mt][:],
                        hs[:, kt, mt * 128 : (mt + 1) * 128],
                        maskn[:, kt, :],
                        start=(kt == 0),
                        stop=(kt == KT_T - 1),
                    )
            for mt in range(MT_D):
                copy(mt, entT[:, mt, :], pool_ps[mt][:])

            # ---- ffnn chains; head (cols 0:EH) first, tail full E ----
            h2T = {}

            def ffnn(side):
                n = EH if side == "h" else E
                h1T = act.tile(
                    [128, KT_H1, n], bf16, tag=f"h1T{side}", name=f"h1T{side}"
                )
                accs = [
                    ps.tile([128, n], f32, tag="mm", name=f"l1{side}{m}")
                    for m in range(MT_H1)
                ]
                for kt in range(KT_MLP):
                    for mt in range(MT_H1):
                        nc.tensor.matmul(
                            accs[mt][:],
                            w1[side][:, kt, mt * 128 : (mt + 1) * 128],
                            entT[:, kt, 0:n],
                            start=(kt == 0),
                            stop=(kt == KT_MLP - 1),
                        )
                for mt in range(MT_H1):
                    nc.scalar.activation(
                        h1T[:, mt, :],
                        accs[mt][:],
                        mybir.ActivationFunctionType.Relu,
                        bias=b1[side][:, mt : mt + 1],
                    )
                h2T[side] = act.tile(
                    [128, KT_H2, n], bf16, tag=f"h2T{side}", name=f"h2T{side}"
                )
                accs2 = [
                    ps.tile([128, n], f32, tag="mm", name=f"l2{side}{m}")
                    for m in range(MT_H2)
                ]
                for kt in range(KT_H1):
                    for mt in range(MT_H2):
                        nc.tensor.matmul(
                            accs2[mt][:],
                            w2[side][:, kt, mt * 128 : (mt + 1) * 128],
                            h1T[:, kt, :],
                            start=(kt == 0),
                            stop=(kt == KT_H1 - 1),
                        )
                for mt in range(MT_H2):
                    nc.scalar.activation(
                        h2T[side][:, mt, :],
                        accs2[mt][:],
                        mybir.ActivationFunctionType.Relu,
                        bias=b2[side][:, mt : mt + 1],
                    )

            ffnn("h")

            # ---- N_o^T [H2, EH] and linh [1, EH] for this core's o ----
            nTo = act.tile([128, KT_H2, EH], bf16, tag="nTo", name="nTo")
            accs = [
                ps.tile([128, EH], f32, tag="mm", name=f"nt{m}") for m in range(MT_H2)
            ]
            for kt in range(KT_H2):
                for mt in range(MT_H2):
                    nc.tensor.matmul(
                        accs[mt][:],
                        wb_o[:, kt, mt * 128 : (mt + 1) * 128],
                        h2T["h"][:, kt, :],
                        start=(kt == 0),
                        stop=(kt == KT_H2 - 1),
                    )
            for mt in range(MT_H2):
                copy(mt, nTo[:, mt, :], accs[mt][:])

            linh = act.tile([1, EH], bf16, tag="linh", name="linh")
            p = ps1.tile([1, EH], f32, tag="lin")
            for kt in range(KT_H2):
                nc.tensor.matmul(
                    p[:],
                    wlin[:, kt, 0:1],
                    h2T["h"][:, kt, :],
                    start=(kt == 0),
                    stop=(kt == KT_H2 - 1),
                )
            nc.vector.tensor_copy(linh[:], p[:])

            ffnn("t")

            lint = act.tile([1, E], bf16, tag="lint", name="lint")
            p = ps1.tile([1, E], f32, tag="lin")
            for kt in range(KT_H2):
                nc.tensor.matmul(
                    p[:],
                    wlin[:, kt, 1:2],
                    h2T["t"][:, kt, :],
                    start=(kt == 0),
                    stop=(kt == KT_H2 - 1),
                )
            # + b_lin[o] folded in via bias
            nc.scalar.activation(
                lint[:],
                p[:],
                mybir.ActivationFunctionType.Identity,
                bias=blin,
            )

            # ---- table rows for this core: [128, E] ----
            slab = act.tile([128, E], bf16, tag="slab")
            p = ps.tile([128, E], f32, tag="mm")
            for kt in range(KT_H2):
                nc.tensor.matmul(
                    p[:],
                    nTo[:, kt, :],
                    h2T["t"][:, kt, :],
                    start=(kt == 0),
                    stop=False,
                )
            nc.tensor.matmul(p[:], linh[:], ones_t[:], start=False, stop=False)
            nc.tensor.matmul(
                p[:], ones_t[:, 0:128], lint[:], start=False, stop=True
            )
            nc.vector.tensor_copy(slab[:], p[:])
            nc.sync.dma_start(d_slab.ap(), slab[:])

    nc.compile()
    _cache[0] = nc
    return nc


def _pack(w, kt, dtype=ml_dtypes.bfloat16):
    """[kt*128, n] row-major -> [128, kt*n] partition-packed."""
    n = w.shape[1]
    return np.ascontiguousarray(
        w.reshape(kt, 128, n).transpose(1, 0, 2).reshape(128, kt * n).astype(dtype)
    )


def _prep_host(inputs):
    """Host-side input packing -> per-core in_maps + assembly info."""
    hs = np.asarray(inputs["hidden_states"], dtype=np.float32)
    start = np.asarray(inputs["entity_start"]).astype(np.int64)
    end = np.asarray(inputs["entity_end"]).astype(np.int64)
    label = np.asarray(inputs["entity_label"]).astype(np.int64)

    t = np.arange(T)
    mask = (
        (t[None, None, :] >= start[:, :, None]) & (t[None, None, :] < end[:, :, None])
    ).astype(np.float32)  # [B,E,T]
    counts = np.maximum(mask.sum(-1, keepdims=True), 1.0)
    masknT = (mask / counts).transpose(0, 2, 1)  # [B,T,E]

    def f32(x):
        return np.ascontiguousarray(np.asarray(x, dtype=np.float32))

    bf = ml_dtypes.bfloat16
    w_bil = f32(inputs["W_bil"])
    w_lin = f32(inputs["W_lin"])
    b_lin = f32(inputs["b_lin"])
    emb_all = f32(inputs["entity_emb_w"])

    shared = {
        "Wh1": _pack(f32(inputs["Wh1"]), KT_MLP),
        "Wt1": _pack(f32(inputs["Wt1"]), KT_MLP),
        "Wh2": _pack(f32(inputs["Wh2"]), KT_H1),
        "Wt2": _pack(f32(inputs["Wt2"]), KT_H1),
    }

    # per-o: Wbil_o | [wlin head col o, wlin tail col o]
    wtl_o = []
    for o in range(OUT):
        wb = _pack(w_bil[o], KT_H2, np.float32)  # [128, 3*384]
        wl = np.stack(
            [
                w_lin[:H2, o].reshape(KT_H2, 128).T,  # [128, 3] head col
                w_lin[H2:, o].reshape(KT_H2, 128).T,  # [128, 3] tail col
            ],
            axis=2,
        ).reshape(128, 2 * KT_H2)
        wtl_o.append(
            np.ascontiguousarray(
                np.concatenate([wb, wl], axis=1).astype(bf)
            )
        )

    smf_o = []
    for o in range(OUT):
        smf = np.zeros((128, SF_COLS), np.float32)
        smf[:, 0:MT_H1] = f32(inputs["bh1"]).reshape(MT_H1, 128).T
        smf[:, MT_H1 : 2 * MT_H1] = f32(inputs["bt1"]).reshape(MT_H1, 128).T
        smf[:, 2 * MT_H1 : 2 * MT_H1 + MT_H2] = (
            f32(inputs["bh2"]).reshape(MT_H2, 128).T
        )
        smf[:, 2 * MT_H1 + MT_H2 : 2 * MT_H1 + 2 * MT_H2] = (
            f32(inputs["bt2"]).reshape(MT_H2, 128).T
        )
        smf[0, SF_COLS - 1] = b_lin[o]
        smf_o.append(smf)

    in_maps = []
    for i in range(N_CORES):
        b, q = divmod(i, 4)
        m, o = divmod(q, 2)
        rot = (np.arange(E) + EH * m) % E  # rotated entity order
        mrot = np.ascontiguousarray(masknT[b][:, rot])
        embT = emb_all[label[b][rot]].T  # [D, E] in rotated order
        embp = np.zeros((128, (MT_D + 1) * E), np.float32)
        embp[:, 0 : MT_D * E] = _pack(embT, MT_D, np.float32)
        embp[0, MT_D * E :] = 1.0  # ones row
        mm = dict(shared)
        mm["hs"] = _pack(hs[b], KT_T)
        mm["masknT"] = _pack(mrot, KT_T)
        mm["embT"] = embp.astype(bf)
        mm["Wtl"] = wtl_o[o]
        mm["smf"] = smf_o[o]
        in_maps.append(mm)

    head_idx = np.asarray(inputs["head_idx"]).astype(np.int64)
    tail_idx = np.asarray(inputs["tail_idx"]).astype(np.int64)
    return in_maps, (head_idx, tail_idx), 0


def kernel(**inputs) -> np.ndarray:
    in_maps, (head_idx, tail_idx), ni = _prep_host(inputs)
    nc = _build(ni)
    res = run_bass_kernel_spmd(nc, in_maps, list(range(N_CORES)))
    out = np.zeros((B, P, OUT), np.float32)
    for b in range(B):
        slabs = np.stack(
            [res.results[4 * b + q]["slab"].astype(np.float32) for q in range(4)]
        )  # [q, 128, E]; q = 2*m + o
        e1, e2 = head_idx[b], tail_idx[b]
        m = e1 // EH
        p_ = e1 % EH
        col = (e2 - EH * m) % E
        for o in range(OUT):
            out[b, :, o] = slabs[2 * m + o, p_, col]
    return out


# revision 13
# speedup vs baseline: 1.1827x; 1.0935x over previous
"""Trainium2 Bass kernel for nn_CellDecoder (span-pool + ffnn + biaffine pairs).

Strategy: head_idx/tail_idx only reference E=256 entities, so instead of
computing the biaffine per pair (P=65536), the cores build the full E x E
biaffine logit table (small matmuls). The per-pair work is a pure table
lookup with host-known indices, done during the host-side unshard/assembly
step, so the device kernel ships the dense table.

Sharding: 8 cores = batch (2) x e1-half (2) x e2-half (2). Each core
computes one 128x128 quadrant of the logit table (for both output logits):
the head ffnn chain runs on its 128 row-entities and the tail chain on its
128 column-entities, so every matmul stage after pooling has free dim 128
with zero cross-core redundancy in the chains. Per-core "which entities"
is steered purely through the inputs: the host packs each core's
mask/embedding columns as [row-subset | column-subset] (the SPMD program
is identical on all cores and just slices 0:128 / 128:256).

Perf notes:
- Everything is bf16; rel err ~5e-3, well under the 2e-2 gate.
- DMA instruction issue costs ~600ns each on a ring, so bulk tensors go
  out as few large transfers on the sync ring in exact consumption order;
  first-needed operands (mask, label-emb rows) ride the scalar ring.
- The label-embedding half of ent_repr is a tiny host-side lookup table
  -> shipped as data straight into the entT tile instead of matmuls.
- Matmul loops are kt-outer over chunk boundaries; psum->sbuf copies
  alternate vector/scalar engines.
"""

import os

os.environ.setdefault("JAX_PLATFORMS", "axon,cpu")

import numpy as np
import ml_dtypes

import concourse.bass as bass
import concourse.tile as tile
from concourse import bacc, mybir
from concourse.bass_utils import run_bass_kernel_spmd

dt = mybir.dt

B, T, D, E, P = 2, 512, 768, 256, 65536
MLP = 2 * D  # 1536
H1, H2 = MLP // 2, MLP // 4  # 768, 384
NL = 5
OUT = 2
N_CORES = 8
EH = 128  # table rows/cols per core (quadrant edge)

KT_MLP = MLP // 128  # 12
KT_H1 = H1 // 128  # 6
KT_H2 = H2 // 128  # 3
KT_T = T // 128  # 4
MT_D = D // 128  # 6
MT_H1 = H1 // 128  # 6
MT_H2 = H2 // 128  # 3

SF_COLS = 2 * MT_H1 + 2 * MT_H2 + OUT  # b1h, b1t, b2h, b2t, blin = 20

_cache: dict = {}


def _build(ni: int = 0):
    """Build + compile the SPMD program (ni unused, kept for test.py interface)."""
    if 0 in _cache:
        return _cache[0]

    nc = bacc.Bacc("TRN2", target_bir_lowering=False, debug=False, num_devices=N_CORES)

    f32 = dt.float32
    bf16 = dt.bfloat16

    # [128, cols] host-packed operand tensors
    d_hs = nc.dram_tensor("hs", [128, KT_T * D], bf16, kind="ExternalInput")
    d_maskn = nc.dram_tensor("masknT", [128, KT_T * E], bf16, kind="ExternalInput")
    # label-emb rows of entT (host-computed lookup) | ones row
    d_emb = nc.dram_tensor("embT", [128, MT_D * E + E], bf16, kind="ExternalInput")
    d_wh1 = nc.dram_tensor("Wh1", [128, KT_MLP * H1], bf16, kind="ExternalInput")
    d_wt1 = nc.dram_tensor("Wt1", [128, KT_MLP * H1], bf16, kind="ExternalInput")
    d_wh2 = nc.dram_tensor("Wh2", [128, KT_H1 * H2], bf16, kind="ExternalInput")
    d_wt2 = nc.dram_tensor("Wt2", [128, KT_H1 * H2], bf16, kind="ExternalInput")
    # Wbil_0 | Wbil_1 | wlin (head/tail cols for both o)
    d_wtl = nc.dram_tensor(
        "Wtl", [128, 2 * KT_H2 * H2 + 2 * KT_H2 * OUT], bf16, kind="ExternalInput"
    )
    d_smf = nc.dram_tensor("smf", [128, SF_COLS], f32, kind="ExternalInput")
    # output: this core's table quadrant per o, columns in subset order
    d_slab = nc.dram_tensor("slab", [128, OUT * EH], bf16, kind="ExternalOutput")

    with tile.TileContext(nc) as tc:
        with (
            tc.tile_pool(name="wbig", bufs=1) as wbig,
            tc.tile_pool(name="wsml", bufs=1) as wsml,
            tc.tile_pool(name="act", bufs=1) as act,
            tc.tile_pool(name="ps", bufs=6, space="PSUM") as ps,
            tc.tile_pool(name="ps1", bufs=2, space="PSUM") as ps1,
        ):
            # ---- first-needed operands on the scalar ring ----
            maskn = wsml.tile([128, KT_T, E], bf16, tag="maskn", name="maskn")
            nc.scalar.dma_start(
                maskn[:], d_maskn.ap().rearrange("p (kt n) -> p kt n", kt=KT_T)
            )
            # entT rows 0:6 filled by pooling below; rows 6:12 + ones by DMA
            entT = act.tile([128, KT_MLP, E], bf16, tag="entT")
            emb_src = d_emb.ap().rearrange("p (kt n) -> p kt n", kt=MT_D + 1)
            nc.scalar.dma_start(entT[:, MT_D:KT_MLP, :], emb_src[:, 0:MT_D, :])
            ones_t = wsml.tile([1, E], bf16, tag="ones", name="ones_t")
            nc.scalar.dma_start(ones_t[:], emb_src[0:1, MT_D, :])
            smf = wsml.tile([128, SF_COLS], f32, tag="smf", name="smf")
            nc.scalar.dma_start(smf[:], d_smf.ap())

            b1 = {"h": smf[:, 0:MT_H1], "t": smf[:, MT_H1 : 2 * MT_H1]}
            b2 = {
                "h": smf[:, 2 * MT_H1 : 2 * MT_H1 + MT_H2],
                "t": smf[:, 2 * MT_H1 + MT_H2 : 2 * MT_H1 + 2 * MT_H2],
            }
            blin = smf[0:1, 2 * MT_H1 + 2 * MT_H2 : SF_COLS]

            # ---- bulk on the sync ring in consumption order ----
            def loadb(pool, name, dram, kt, n, nchunks=1):
                t = pool.tile([128, kt, n], bf16, tag=name, name=name)
                src = dram.ap().rearrange("p (kt n) -> p kt n", kt=kt)
                step = kt // nchunks
                for k0 in range(0, kt, step):
                    nc.sync.dma_start(t[:, k0 : k0 + step, :], src[:, k0 : k0 + step, :])
                return t

            hs = loadb(wbig, "hs", d_hs, KT_T, D, nchunks=2)
            w1 = {"h": loadb(wbig, "w1h", d_wh1, KT_MLP, H1, nchunks=2)}
            w2 = {"h": loadb(wbig, "w2h", d_wh2, KT_H1, H2)}
            wtl = wsml.tile([128, KT_H2, 2 * H2 + 2 * OUT], bf16, tag="wtl", name="wtl")
            nc.sync.dma_start(
                wtl[:, :, 0 : 2 * H2],
                d_wtl.ap()[:, 0 : 2 * KT_H2 * H2].rearrange(
                    "p (kt n) -> p kt n", kt=KT_H2
                ),
            )
            nc.sync.dma_start(
                wtl[:, :, 2 * H2 :],
                d_wtl.ap()[:, 2 * KT_H2 * H2 :].rearrange(
                    "p (kt n) -> p kt n", kt=KT_H2
                ),
            )
            wb = [wtl[:, :, 0:H2], wtl[:, :, H2 : 2 * H2]]
            # per o: [:, kt, 2*H2 + 2*o] = head col, [:, kt, 2*H2 + 2*o + 1] = tail col
            w1["t"] = loadb(wbig, "w1t", d_wt1, KT_MLP, H1, nchunks=2)
            w2["t"] = loadb(wbig, "w2t", d_wt2, KT_H1, H2)

            # copy engines alternate to halve serial copy chains
            def copy(i, dst, src):
                if i % 2:
                    nc.scalar.activation(
                        dst, src, mybir.ActivationFunctionType.Identity
                    )
                else:
                    nc.vector.tensor_copy(dst, src)

            # ---- pooled^T -> entT rows 0:6  (kt-outer over hs chunks) ----
            pool_ps = [
                ps.tile([128, E], f32, tag="mm", name=f"pp{m}") for m in range(MT_D)
            ]
            for kt in range(KT_T):
                for mt in range(MT_D):
                    nc.tensor.matmul(
                        pool_ps[mt][:],
                        hs[:, kt, mt * 128 : (mt + 1) * 128],
                        maskn[:, kt, :],
                        start=(kt == 0),
                        stop=(kt == KT_T - 1),
                    )
            for mt in range(MT_D):
                copy(mt, entT[:, mt, :], pool_ps[mt][:])

            # ---- ffnn chains on 128-entity subsets:
            #      head = entT cols 0:128, tail = cols 128:256 ----
            h2T = {}

            def ffnn(side):
                lo = 0 if side == "h" else EH
                h1T = act.tile(
                    [128, KT_H1, EH], bf16, tag=f"h1T{side}", name=f"h1T{side}"
                )
                accs = [
                    ps.tile([128, EH], f32, tag="mm", name=f"l1{side}{m}")
                    for m in range(MT_H1)
                ]
                for kt in range(KT_MLP):
                    for mt in range(MT_H1):
                        nc.tensor.matmul(
                            accs[mt][:],
                            w1[side][:, kt, mt * 128 : (mt + 1) * 128],
                            entT[:, kt, lo : lo + EH],
                            start=(kt == 0),
                            stop=(kt == KT_MLP - 1),
                        )
                for mt in range(MT_H1):
                    nc.scalar.activation(
                        h1T[:, mt, :],
                        accs[mt][:],
                        mybir.ActivationFunctionType.Relu,
                        bias=b1[side][:, mt : mt + 1],
                    )
                h2T[side] = act.tile(
                    [128, KT_H2, EH], bf16, tag=f"h2T{side}", name=f"h2T{side}"
                )
                accs2 = [
                    ps.tile([128, EH], f32, tag="mm", name=f"l2{side}{m}")
                    for m in range(MT_H2)
                ]
                for kt in range(KT_H1):
                    for mt in range(MT_H2):
                        nc.tensor.matmul(
                            accs2[mt][:],
                            w2[side][:, kt, mt * 128 : (mt + 1) * 128],
                            h1T[:, kt, :],
                            start=(kt == 0),
                            stop=(kt == KT_H1 - 1),
                        )
                for mt in range(MT_H2):
                    nc.scalar.activation(
                        h2T[side][:, mt, :],
                        accs2[mt][:],
                        mybir.ActivationFunctionType.Relu,
                        bias=b2[side][:, mt : mt + 1],
                    )

            ffnn("h")

            # ---- N_o^T [H2, EH] and linh [1, EH] for both o ----
            nT = []
            linh = []
            for o in range(OUT):
                nTo = act.tile([128, KT_H2, EH], bf16, tag=f"nT{o}", name=f"nT{o}")
                accs = [
                    ps.tile([128, EH], f32, tag="mm", name=f"nt{o}{m}")
                    for m in range(MT_H2)
                ]
                for kt in range(KT_H2):
                    for mt in range(MT_H2):
                        nc.tensor.matmul(
                            accs[mt][:],
                            wb[o][:, kt, mt * 128 : (mt + 1) * 128],
                            h2T["h"][:, kt, :],
                            start=(kt == 0),
                            stop=(kt == KT_H2 - 1),
                        )
                for mt in range(MT_H2):
                    copy(mt, nTo[:, mt, :], accs[mt][:])
                nT.append(nTo)

                lh = act.tile([1, EH], bf16, tag=f"linh{o}", name=f"linh{o}")
                p = ps1.tile([1, EH], f32, tag="lin")
                for kt in range(KT_H2):
                    nc.tensor.matmul(
                        p[:],
                        wtl[:, kt, 2 * H2 + 2 * o : 2 * H2 + 2 * o + 1],
                        h2T["h"][:, kt, :],
                        start=(kt == 0),
                        stop=(kt == KT_H2 - 1),
                    )
                nc.vector.tensor_copy(lh[:], p[:])
                linh.append(lh)

            ffnn("t")

            lint = []
            for o in range(OUT):
                lt = act.tile([1, EH], bf16, tag=f"lint{o}", name=f"lint{o}")
                p = ps1.tile([1, EH], f32, tag="lin")
                for kt in range(KT_H2):
                    nc.tensor.matmul(
                        p[:],
                        wtl[:, kt, 2 * H2 + 2 * o + 1 : 2 * H2 + 2 * o + 2],
                        h2T["t"][:, kt, :],
                        start=(kt == 0),
                        stop=(kt == KT_H2 - 1),
                    )
                # + b_lin[o] folded in via bias
                nc.scalar.activation(
                    lt[:],
                    p[:],
                    mybir.ActivationFunctionType.Identity,
                    bias=blin[:, o : o + 1],
                )
                lint.append(lt)

            # ---- table quadrant for this core: [128, OUT, EH] ----
            slab = act.tile([128, OUT, EH], bf16, tag="slab")
            for o in range(OUT):
                p = ps.tile([128, EH], f32, tag="mm")
                for kt in range(KT_H2):
                    nc.tensor.matmul(
                        p[:],
                        nT[o][:, kt, :],
                        h2T["t"][:, kt, :],
                        start=(kt == 0),
                        stop=False,
                    )
                nc.tensor.matmul(
                    p[:], linh[o][:], ones_t[:, 0:EH], start=False, stop=False
                )
                nc.tensor.matmul(
                    p[:], ones_t[:, 0:128], lint[o][:], start=False, stop=True
                )
                copy(o, slab[:, o, :], p[:])
            nc.sync.dma_start(
                d_slab.ap().rearrange("p (o n) -> p o n", o=OUT), slab[:]
            )

    nc.compile()
    _cache[0] = nc
    return nc


def _pack(w, kt, dtype=ml_dtypes.bfloat16):
    """[kt*128, n] row-major -> [128, kt*n] partition-packed."""
    n = w.shape[1]
    return np.ascontiguousarray(
        w.reshape(kt, 128, n).transpose(1, 0, 2).reshape(128, kt * n).astype(dtype)
    )


def _prep_host(inputs):
    """Host-side input packing -> per-core in_maps + assembly info."""
    hs = np.asarray(inputs["hidden_states"], dtype=np.float32)
    start = np.asarray(inputs["entity_start"]).astype(np.int64)
    end = np.asarray(inputs["entity_end"]).astype(np.int64)
    label = np.asarray(inputs["entity_label"]).astype(np.int64)

    t = np.arange(T)
    mask = (
        (t[None, None, :] >= start[:, :, None]) & (t[None, None, :] < end[:, :, None])
    ).astype(np.float32)  # [B,E,T]
    counts = np.maximum(mask.sum(-1, keepdims=True), 1.0)
    masknT = (mask / counts).transpose(0, 2, 1)  # [B,T,E]

    def f32(x):
        return np.ascontiguousarray(np.asarray(x, dtype=np.float32))

    bf = ml_dtypes.bfloat16
    w_bil = f32(inputs["W_bil"])
    w_lin = f32(inputs["W_lin"])
    b_lin = f32(inputs["b_lin"])
    emb_all = f32(inputs["entity_emb_w"])

    # Wbil_0 | Wbil_1 | wlin cols interleaved per o as [head_o, tail_o]
    wb0 = _pack(w_bil[0], KT_H2, np.float32).reshape(128, KT_H2, H2)
    wb1 = _pack(w_bil[1], KT_H2, np.float32).reshape(128, KT_H2, H2)
    wl = np.stack(
        [
            w_lin[:H2, 0].reshape(KT_H2, 128).T,
            w_lin[H2:, 0].reshape(KT_H2, 128).T,
            w_lin[:H2, 1].reshape(KT_H2, 128).T,
            w_lin[H2:, 1].reshape(KT_H2, 128).T,
        ],
        axis=2,
    )  # [128, KT_H2, 4]
    # device reads region1 = cols 0:2*KT_H2*H2 as [kt, 2*H2], region2 = rest
    # as [kt, 2*OUT] -> pack the wb pair kt-major first, then wl kt-major
    region1 = np.concatenate([wb0, wb1], axis=2).reshape(128, -1)
    region2 = wl.reshape(128, -1)
    wtl = np.ascontiguousarray(
        np.concatenate([region1, region2], axis=1).astype(bf)
    )

    smf = np.zeros((128, SF_COLS), np.float32)
    smf[:, 0:MT_H1] = f32(inputs["bh1"]).reshape(MT_H1, 128).T
    smf[:, MT_H1 : 2 * MT_H1] = f32(inputs["bt1"]).reshape(MT_H1, 128).T
    smf[:, 2 * MT_H1 : 2 * MT_H1 + MT_H2] = f32(inputs["bh2"]).reshape(MT_H2, 128).T
    smf[:, 2 * MT_H1 + MT_H2 : 2 * MT_H1 + 2 * MT_H2] = (
        f32(inputs["bt2"]).reshape(MT_H2, 128).T
    )
    smf[0, 2 * MT_H1 + 2 * MT_H2 : SF_COLS] = b_lin

    shared = {
        "Wh1": _pack(f32(inputs["Wh1"]), KT_MLP),
        "Wt1": _pack(f32(inputs["Wt1"]), KT_MLP),
        "Wh2": _pack(f32(inputs["Wh2"]), KT_H1),
        "Wt2": _pack(f32(inputs["Wt2"]), KT_H1),
        "Wtl": wtl,
        "smf": smf,
    }

    in_maps = []
    for i in range(N_CORES):
        b, q = divmod(i, 4)
        r, c = divmod(q, 2)  # row-half, col-half of the table quadrant
        sel = np.concatenate(
            [np.arange(EH * r, EH * r + EH), np.arange(EH * c, EH * c + EH)]
        )  # [head subset | tail subset]
        mrot = np.ascontiguousarray(masknT[b][:, sel])
        embT = emb_all[label[b][sel]].T  # [D, E] in subset order
        embp = np.zeros((128, (MT_D + 1) * E), np.float32)
        embp[:, 0 : MT_D * E] = _pack(embT, MT_D, np.float32)
        embp[0, MT_D * E :] = 1.0  # ones row
        mm = dict(shared)
        mm["hs"] = _pack(hs[b], KT_T)
        mm["masknT"] = _pack(mrot, KT_T)
        mm["embT"] = embp.astype(bf)
        in_maps.append(mm)

    head_idx = np.asarray(inputs["head_idx"]).astype(np.int64)
    tail_idx = np.asarray(inputs["tail_idx"]).astype(np.int64)
    return in_maps, (head_idx, tail_idx), 0


def kernel(**inputs) -> np.ndarray:
    in_maps, (head_idx, tail_idx), ni = _prep_host(inputs)
    nc = _build(ni)
    res = run_bass_kernel_spmd(nc, in_maps, list(range(N_CORES)))
    out = np.zeros((B, P, OUT), np.float32)
    for b in range(B):
        slabs = np.stack(
            [
                res.results[4 * b + q]["slab"].reshape(128, OUT, EH).astype(np.float32)
                for q in range(4)
            ]
        )  # [q, 128, OUT, EH]; q = 2*r + c
        e1, e2 = head_idx[b], tail_idx[b]
        q = 2 * (e1 // EH) + (e2 // EH)
        out[b] = slabs[q, e1 % EH, :, e2 % EH]
    return out
